# revision 20
# baseline (speedup 1.0000x reference)
"""Trainium2 Bass kernel for nn_DCGAN_G (DCGAN generator + 69-neuron spiking scan).

Strategy (8 NeuronCores, SPMD):
  A. W_in matvec (12800x2048) row-sharded 8x -> AllGather h1 (12800).
  B. DCGAN conv stack replicated on every core (tiny: ~3 GMAC).
  C. W_d2 matvec (4761x6400) row-sharded 8x -> AllGather w (69x69).
  D. 99800-step spiking recurrence (inherently serial), restructured to
     a 2-engine critical cycle in "z-space": with z_t = (-1)^t s_t @ w,
     the step  s_{t+1} = tanh(s_t@w) - s_t  becomes
         z_{t+1} = z_t - tanh(z_t) @ w
     i.e. one PE matmul accumulating -w^T v into a single persistent
     PSUM bank + one ACT tanh per step (267.7 ns critical cycle:
     173 PE write pipeline + 94 sem hops), vs the naive 3-engine
     {matmul, tanh, subtract} chain at 315 ns. The output states follow
     from the shadow chain ss_{t+1} = ss_t - v_t (ss_t = (-1)^t s_t) on
     the otherwise-idle DVE, off the critical path. (69,B) state blocks
     are PE-transposed to row-major inline (hidden under the scan), the
     (-1)^(r+1) row sign is folded into the per-partition scalar of the
     post-transpose copy, and outputs go to DRAM in 2 strided DMAs per
     998-step block (<=16 dynamic-DMA registers per hardware loop
     iteration; the For_i unrolls 5 blocks = 4990 steps per iteration).
"""
import numpy as np

import bass_rust
import concourse.bass as bass
import concourse.mybir as mybir
from concourse.bass_utils import run_bass_kernel_spmd
from concourse.tile import TileContext
from concourse.vector_clock import ScopedClock

f32 = mybir.dt.float32
AF = mybir.ActivationFunctionType
OP = mybir.AluOpType
AX = mybir.AxisListType

T_FULL = 99800
N = 69
NCORES = 8
EPS = 1e-5
SCAN_B = 998          # even (sign-folding needs even block starts)
SCAN_U = 5            # unrolled blocks per For_i iteration (4990 steps/iter)
MROWS_A = 1600        # W_in rows per core
MROWS_C = 596         # W_d2 rows per core (8*596=4768 >= 4761)


# ---------------------------------------------------------------------------
# walrus workaround: CTRL-type instructions accept at most 1 sem wait, but the
# TileContext tail drain gets one wait per active proc. Split across drains.
def _patched_drain_and_barrier(self, tick_clock, wait_clock):
    drain_inst = self.nc.sync.drain()
    wait_clock.add_sem_waits(
        drain_inst.ins, ScopedClock({None: tick_clock.global_clock})
    )
    si = drain_inst.ins.sync_info
    waits = list(si.on_wait) if si is not None else []
    if len(waits) > 1:
        drain_inst.ins.sync_info = bass_rust.SyncInfo(
            on_wait=waits[:1], on_update=list(si.on_update)
        )
        for i in range(1, len(waits)):
            extra = self.nc.sync.drain()
            extra.ins.sync_info = bass_rust.SyncInfo(
                on_wait=waits[i : i + 1], on_update=[]
            )
    self.nc.all_engine_barrier()
    assert self.sems is not None
    popped = self.nc._tile_sem_poison_stack.pop()
    assert popped is self._sem_poison
    self.nc.clear_and_free_semaphores(list(self.sems.allocated().values()))
    self.nc.all_engine_barrier()


TileContext._drain_and_barrier = _patched_drain_and_barrier
# ---------------------------------------------------------------------------


def _split_excess_waits(nc, max_waits=1):
    """This walrus build accepts at most one sem wait per instruction; move
    excess waits onto single-wait NOPs inserted just before the owner."""
    n_split = 0
    for f in nc.m.functions:
        for b in f.blocks:
            insts = list(b.instructions)
            out = []
            changed = False
            for inst in insts:
                si = inst.sync_info
                waits = list(si.on_wait) if si is not None else []
                if len(waits) > max_waits:
                    changed = True
                    for i, w in enumerate(waits[max_waits:]):
                        nop = mybir.InstNoOp(
                            name=f"wsp_{inst.name}_{i}", ins=[], outs=[])
                        nop.engine = inst.engine
                        nop.sync_info = bass_rust.SyncInfo(
                            on_wait=[w], on_update=[])
                        out.append(nop)
                        n_split += 1
                    inst.sync_info = bass_rust.SyncInfo(
                        on_wait=waits[:max_waits], on_update=list(si.on_update))
                out.append(inst)
            if changed:
                b.instructions = out
    return n_split


def _pad_w5(w5):
    """(1,64,4,4) -> (4,4,64,32) with real weights in out-column 0."""
    t = np.zeros((4, 4, 64, 32), np.float32)
    t[:, :, :, 0:1] = w5.transpose(2, 3, 1, 0)
    return np.ascontiguousarray(t)


def _col_major_pad(v, ncols):
    """(n,) -> (128, ncols) with element m at [m % 128, m // 128], zero pad."""
    out = np.zeros(128 * ncols, np.float32)
    out[: v.shape[0]] = v
    return np.ascontiguousarray(out.reshape(ncols, 128).T)


def build_program(T=T_FULL, with_front=True, with_scan=True):
    nc = bass.Bass()
    scan_u = SCAN_U if T % (SCAN_B * SCAN_U) == 0 else 1
    assert T % (SCAN_B * scan_u) == 0, "T must be a multiple of the block"

    # ---- inputs ----
    x_cols = nc.declare_dram_parameter("x_cols", [128, 16], f32, isOutput=False)
    win_t = nc.declare_dram_parameter("win_t", [2048, MROWS_A], f32, isOutput=False)
    bin_c = nc.declare_dram_parameter("bin_c", [128, 13], f32, isOutput=False)
    w1c = nc.declare_dram_parameter("w1c", [4, 4, 512, 64], f32, isOutput=False)
    w2c = nc.declare_dram_parameter("w2c", [4, 4, 512, 32], f32, isOutput=False)
    g1c_in = nc.declare_dram_parameter("g1c", [64, 1], f32, isOutput=False)
    be1c_in = nc.declare_dram_parameter("be1c", [64, 1], f32, isOutput=False)
    g2c_in = nc.declare_dram_parameter("g2c", [32, 1], f32, isOutput=False)
    be2c_in = nc.declare_dram_parameter("be2c", [32, 1], f32, isOutput=False)
    w3t = nc.declare_dram_parameter("w3t", [4, 4, 256, 128], f32, isOutput=False)
    w4t = nc.declare_dram_parameter("w4t", [4, 4, 128, 64], f32, isOutput=False)
    w5t = nc.declare_dram_parameter("w5t", [4, 4, 64, 32], f32, isOutput=False)
    g_all = nc.declare_dram_parameter("g_all", [128, 8], f32, isOutput=False)
    be_all = nc.declare_dram_parameter("be_all", [128, 8], f32, isOutput=False)
    wd2_t = nc.declare_dram_parameter("wd2_t", [6400, MROWS_C], f32, isOutput=False)
    bd2_c = nc.declare_dram_parameter("bd2_c", [128, 5], f32, isOutput=False)
    s0_in = nc.declare_dram_parameter("s0", [N, 1], f32, isOutput=False)
    ident_in = nc.declare_dram_parameter("ident", [128, 128], f32, isOutput=False)
    sgn_in = nc.declare_dram_parameter("sgn", [128, 1], f32, isOutput=False)
    if with_scan:
        out_traj = nc.declare_dram_parameter("out", [T, N], f32, isOutput=True)
    else:
        w_out = nc.declare_dram_parameter("w_out", [N, N], f32, isOutput=True)

    # ---- internal DRAM ----
    h_shard = nc.dram_tensor("h_shard", [MROWS_A], f32)
    h_full = nc.dram_tensor("h_full", [NCORES * MROWS_A], f32, addr_space="Shared")
    l1_shard = nc.dram_tensor("l1_shard", [64 * 100], f32)
    l1_full = nc.dram_tensor("l1_full", [512 * 100], f32, addr_space="Shared")
    l2_shard = nc.dram_tensor("l2_shard", [32 * 400], f32)
    l2_full = nc.dram_tensor("l2_full", [256 * 400], f32, addr_space="Shared")
    c_scr = nc.dram_tensor("c_scr", [32, 6400], f32)
    wd_shard = nc.dram_tensor("wd_shard", [MROWS_C], f32)
    w_full = nc.dram_tensor("w_full", [NCORES * MROWS_C], f32, addr_space="Shared")

    with TileContext(nc) as tc:
        # ================= Phase A: h = W_in @ x + b_in (sharded) ==========
        with (
            tc.tile_pool(name="a_const", bufs=1) as acp,
            tc.tile_pool(name="a_slab", bufs=2) as asp,
            tc.tile_pool(name="a_ps", bufs=1, space="PSUM") as aps,
        ):
            xc = acp.tile([128, 16], f32)
            nc.sync.dma_start(out=xc[:, :], in_=x_cols[:, :])
            bc = acp.tile([128, 13], f32)
            nc.sync.dma_start(out=bc[:, :], in_=bin_c[:, :])
            hc = acp.tile([128, 13], f32)
            for jlo, jhi in ((0, 8), (8, 13)):
                ptiles = {}
                for j in range(jlo, jhi):
                    pt = aps.tile([128, 1], f32, tag=f"hps{j - jlo}", name=f"hps{j}")
                    ptiles[j] = pt
                for k in range(16):
                    gw = min(128 * jhi, MROWS_A) - 128 * jlo
                    slab = asp.tile([128, 1024], f32, tag="aslab")
                    nc.sync.dma_start(
                        out=slab[:, :gw],
                        in_=win_t[128 * k : 128 * (k + 1),
                                  128 * jlo : 128 * jlo + gw])
                    for j in range(jlo, jhi):
                        cj = 128 if j < 12 else 64
                        jj = j - jlo
                        nc.tensor.matmul(
                            ptiles[j][:cj, :],
                            slab[:, 128 * jj : 128 * jj + cj],
                            xc[:, k : k + 1],
                            start=(k == 0),
                            stop=(k == 15),
                        )
                for j in range(jlo, jhi):
                    cj = 128 if j < 12 else 64
                    nc.vector.tensor_tensor(
                        out=hc[:cj, j : j + 1], in0=ptiles[j][:cj, :],
                        in1=bc[:cj, j : j + 1], op=OP.add)
            for j in range(13):
                cj = 128 if j < 12 else 64
                nc.sync.dma_start(
                    out=h_shard[128 * j : 128 * j + cj], in_=hc[:cj, j])
        nc.gpsimd.collective_compute(
            "AllGather", OP.bypass, replica_groups=[list(range(NCORES))],
            ins=[h_shard[:]], outs=[h_full[:]])

        # ====== Phase B: conv stack (L1/L2 channel-sharded, rest replicated)
        _lvl = 9  # all conv layers (bisection gates left in place, fully on)
        h2d = h_full.rearrange("(c hw) -> c hw", hw=25)

        with (
            tc.tile_pool(name="bn_const", bufs=1) as bnp,
            tc.tile_pool(name="conv_ps", bufs=1, space="PSUM") as bps,
        ):
            g_sb = bnp.tile([128, 8], f32)
            nc.sync.dma_start(out=g_sb[:, :], in_=g_all[:, :])
            be_sb = bnp.tile([128, 8], f32)
            nc.sync.dma_start(out=be_sb[:, :], in_=be_all[:, :])
            g1c_sb = bnp.tile([64, 1], f32)
            nc.sync.dma_start(out=g1c_sb[:, :], in_=g1c_in[:, :])
            be1c_sb = bnp.tile([64, 1], f32)
            nc.sync.dma_start(out=be1c_sb[:, :], in_=be1c_in[:, :])
            g2c_sb = bnp.tile([32, 1], f32)
            nc.sync.dma_start(out=g2c_sb[:, :], in_=g2c_in[:, :])
            be2c_sb = bnp.tile([32, 1], f32)
            nc.sync.dma_start(out=be2c_sb[:, :], in_=be2c_in[:, :])

            def bn_relu(raw, hw, cch, g_ap, be_ap, out_ap, name):
                """BatchNorm(train) + ReLU from raw (cch,hw) into out_ap."""
                with tc.tile_pool(name=f"bn{name}", bufs=1) as p:
                    s1 = p.tile([cch, 1], f32, tag="s1")
                    nc.vector.tensor_reduce(s1[:, :], raw, axis=AX.X, op=OP.add)
                    mean = p.tile([cch, 1], f32, tag="mean")
                    nc.vector.tensor_scalar_mul(mean[:, :], s1[:, :], 1.0 / hw)
                    sq = p.tile([cch, hw], f32, tag="sq")
                    nc.vector.tensor_tensor(out=sq[:, :], in0=raw, in1=raw, op=OP.mult)
                    s2 = p.tile([cch, 1], f32, tag="s2")
                    nc.vector.tensor_reduce(s2[:, :], sq[:, :], axis=AX.X, op=OP.add)
                    ex2 = p.tile([cch, 1], f32, tag="ex2")
                    nc.vector.tensor_scalar_mul(ex2[:, :], s2[:, :], 1.0 / hw)
                    msq = p.tile([cch, 1], f32, tag="msq")
                    nc.vector.tensor_tensor(
                        out=msq[:, :], in0=mean[:, :], in1=mean[:, :], op=OP.mult)
                    var = p.tile([cch, 1], f32, tag="var")
                    nc.vector.tensor_tensor(
                        out=var[:, :], in0=ex2[:, :], in1=msq[:, :], op=OP.subtract)
                    vps = p.tile([cch, 1], f32, tag="vps")
                    nc.vector.tensor_scalar_add(vps[:, :], var[:, :], EPS)
                    sd = p.tile([cch, 1], f32, tag="sd")
                    nc.scalar.activation(sd[:, :], vps[:, :], AF.Sqrt)
                    rstd = p.tile([cch, 1], f32, tag="rstd")
                    nc.vector.reciprocal(rstd[:, :], sd[:, :])
                    scale = p.tile([cch, 1], f32, tag="scale")
                    nc.vector.tensor_tensor(
                        out=scale[:, :], in0=g_ap, in1=rstd[:, :], op=OP.mult)
                    t1 = p.tile([cch, 1], f32, tag="t1")
                    nc.vector.tensor_tensor(
                        out=t1[:, :], in0=mean[:, :], in1=scale[:, :], op=OP.mult)
                    bia = p.tile([cch, 1], f32, tag="bia")
                    nc.vector.tensor_tensor(
                        out=bia[:, :], in0=be_ap, in1=t1[:, :], op=OP.subtract)
                    nc.scalar.activation(
                        out_ap, raw, AF.Relu, bias=bia[:, :], scale=scale[:, :])

            # ---- L1: up2(h:512x5x5)->512x10x10 conv 512->512 ----
            with (
                tc.tile_pool(name="l1_in", bufs=1) as l1i,
                tc.tile_pool(name="l1_w", bufs=2) as l1w,
                tc.tile_pool(name="l1_out", bufs=1) as l1o,
            ):
                pads1 = []
                for j in range(4):
                    hm = l1i.tile([128, 25], f32, tag=f"hm{j}")
                    nc.sync.dma_start(out=hm[:, :], in_=h2d[128 * j : 128 * (j + 1), :])
                    pad = l1i.tile([128, 13 * 13], f32, tag=f"pad1_{j}")
                    nc.vector.memset(pad[:, :], 0.0)
                    pv = pad[:, :].rearrange("c (h w) -> c h w", h=13)
                    hv = hm[:, :].rearrange("c (h w) -> c h w", h=5)
                    for a in range(2):
                        for b in range(2):
                            nc.vector.tensor_copy(
                                pv[:, a + 1 : a + 11 : 2, b + 1 : b + 11 : 2], hv[:, :, :])
                    pads1.append(pad)
                ps1 = bps.tile([64, 100], f32, tag="l1ps", name="l1ps")
                nmm = 0
                for ji in range(4):
                    for dy in range(4):
                        for dx in range(4):
                            slab = l1w.tile([128, 64], f32, tag="w1slab")
                            nc.sync.dma_start(
                                out=slab[:, :],
                                in_=w1c[dy, dx, 128 * ji : 128 * (ji + 1), :])
                            rhs = pads1[ji][:, :].rearrange(
                                "c (h w) -> c h w", h=13)[:, dy : dy + 10, dx : dx + 10]
                            nc.tensor.matmul(
                                ps1[:, :], slab[:, :], rhs,
                                start=(nmm == 0), stop=(nmm == 63))
                            nmm += 1
                raw = l1o.tile([64, 100], f32, tag="raw1")
                nc.vector.tensor_copy(raw[:, :], ps1[:, :])
                relu = l1o.tile([64, 100], f32, tag="relu1")
                bn_relu(raw[:, :], 100, 64, g1c_sb[:, :], be1c_sb[:, :],
                        relu[:, :], "1")
                nc.sync.dma_start(
                    out=l1_shard[:].rearrange("(c hw) -> c hw", hw=100),
                    in_=relu[:, :])
                nc.gpsimd.collective_compute(
                    "AllGather", OP.bypass,
                    replica_groups=[list(range(NCORES))],
                    ins=[l1_shard[:]], outs=[l1_full[:]])
                l1_2d = l1_full.rearrange("(c hw) -> c hw", hw=100)
                pads2 = []
                for jo in range(4):
                    rm = l1o.tile([128, 100], f32, tag=f"rm1_{jo}")
                    nc.sync.dma_start(
                        out=rm[:, :], in_=l1_2d[128 * jo : 128 * (jo + 1), :])
                    pad = l1o.tile([128, 23 * 23], f32, tag=f"pad2_{jo}")
                    nc.vector.memset(pad[:, :], 0.0)
                    pv = pad[:, :].rearrange("c (h w) -> c h w", h=23)
                    rv = rm[:, :].rearrange("c (h w) -> c h w", h=10)
                    for a in range(2):
                        for b in range(2):
                            nc.vector.tensor_copy(
                                pv[:, a + 1 : a + 21 : 2, b + 1 : b + 21 : 2], rv[:, :, :])
                    pads2.append(pad)

                if _lvl >= 2:
                  # ---- L2: 512x20x20 conv 512->256 ----
                  with (
                      tc.tile_pool(name="l2_w", bufs=2) as l2w,
                      tc.tile_pool(name="l2_out", bufs=1) as l2o,
                  ):
                      psA = bps.tile([32, 400], f32, tag="cpsA")
                      nmm = 0
                      for ji in range(4):
                          for dy in range(4):
                              for dx in range(4):
                                  slab = l2w.tile([128, 32], f32, tag="w2slab")
                                  nc.sync.dma_start(
                                      out=slab[:, :],
                                      in_=w2c[dy, dx, 128 * ji : 128 * (ji + 1), :])
                                  rhs = pads2[ji][:, :].rearrange(
                                      "c (h w) -> c h w", h=23)[:, dy : dy + 20, dx : dx + 20]
                                  nc.tensor.matmul(
                                      psA[:, :], slab[:, :], rhs,
                                      start=(nmm == 0), stop=(nmm == 63))
                                  nmm += 1
                      raw2 = l2o.tile([32, 400], f32, tag="raw2")
                      nc.vector.tensor_copy(raw2[:, :], psA[:, :])
                      relu2 = l2o.tile([32, 400], f32, tag="relu2")
                      bn_relu(raw2[:, :], 400, 32, g2c_sb[:, :], be2c_sb[:, :],
                              relu2[:, :], "2")
                      nc.sync.dma_start(
                          out=l2_shard[:].rearrange("(c hw) -> c hw", hw=400),
                          in_=relu2[:, :])
                      nc.gpsimd.collective_compute(
                          "AllGather", OP.bypass,
                          replica_groups=[list(range(NCORES))],
                          ins=[l2_shard[:]], outs=[l2_full[:]])
                      l2_2d = l2_full.rearrange("(c hw) -> c hw", hw=400)
                      pads3 = []
                      for jo in range(2):
                          rm2 = l2o.tile([128, 400], f32, tag=f"rm2_{jo}")
                          nc.sync.dma_start(
                              out=rm2[:, :],
                              in_=l2_2d[128 * jo : 128 * (jo + 1), :])
                          pad = l2o.tile([128, 43 * 43], f32, tag=f"pad3_{jo}")
                          nc.vector.memset(pad[:, :], 0.0)
                          pv = pad[:, :].rearrange("c (h w) -> c h w", h=43)
                          rv = rm2[:, :].rearrange("c (h w) -> c h w", h=20)
                          for a in range(2):
                              for b in range(2):
                                  nc.vector.tensor_copy(
                                      pv[:, a + 1 : a + 41 : 2, b + 1 : b + 41 : 2],
                                      rv[:, :, :])
                          pads3.append(pad)

                      if _lvl >= 3:
                        # ---- L3: 256x40x40 conv 256->128 ----
                        with (
                            tc.tile_pool(name="l3_w", bufs=1) as l3w,
                            tc.tile_pool(name="l3_out", bufs=1) as l3o,
                        ):
                            wsl3 = l3w.tile([128, 32 * 128], f32)
                            for ji in range(2):
                                for dy in range(4):
                                    for dx in range(4):
                                        si = (ji * 16 + dy * 4 + dx) * 128
                                        nc.sync.dma_start(
                                            out=wsl3[:, si : si + 128],
                                            in_=w3t[dy, dx, 128 * ji : 128 * (ji + 1), :])
                            raw3 = l3o.tile([128, 1600], f32)
                            for st in range(4):
                                ps = bps.tile([128, 400], f32, tag="cps", bufs=2)
                                nmm = 0
                                for ji in range(2):
                                    for dy in range(4):
                                        for dx in range(4):
                                            si = (ji * 16 + dy * 4 + dx) * 128
                                            rhs = pads3[ji][:, :].rearrange(
                                                "c (h w) -> c h w", h=43)[
                                                :, st * 10 + dy : st * 10 + dy + 10,
                                                dx : dx + 40]
                                            nc.tensor.matmul(
                                                ps[:, :], wsl3[:, si : si + 128], rhs,
                                                start=(nmm == 0), stop=(nmm == 31))
                                            nmm += 1
                                nc.vector.tensor_copy(
                                    raw3[:, 400 * st : 400 * (st + 1)], ps[:, :])
                            relu3 = l3o.tile([128, 1600], f32)
                            bn_relu(raw3[:, :], 1600, 128, g_sb[:128, 6:7],
                                    be_sb[:128, 6:7], relu3[:, :], "3")
                            pad4 = l3o.tile([128, 83 * 83], f32)
                            nc.vector.memset(pad4[:, :], 0.0)
                            pv = pad4[:, :].rearrange("c (h w) -> c h w", h=83)
                            rv = relu3[:, :].rearrange("c (h w) -> c h w", h=40)
                            for a in range(2):
                                for b in range(2):
                                    nc.vector.tensor_copy(
                                        pv[:, a + 1 : a + 81 : 2, b + 1 : b + 81 : 2],
                                        rv[:, :, :])

                            if _lvl >= 4:
                              # ---- L4: 128x80x80 conv 128->64 ----
                              with (
                                  tc.tile_pool(name="l4_w", bufs=1) as l4w,
                                  tc.tile_pool(name="l4_out", bufs=1) as l4o,
                              ):
                                  wsl4 = l4w.tile([128, 16 * 64], f32)
                                  for dy in range(4):
                                      for dx in range(4):
                                          si = (dy * 4 + dx) * 64
                                          nc.sync.dma_start(
                                              out=wsl4[:, si : si + 64],
                                              in_=w4t[dy, dx, :, :])
                                  raw4 = l4o.tile([64, 6400], f32)
                                  for st in range(16):
                                      ps = bps.tile([64, 400], f32, tag="cps", bufs=2)
                                      nmm = 0
                                      for dy in range(4):
                                          for dx in range(4):
                                              si = (dy * 4 + dx) * 64
                                              rhs = pad4[:, :].rearrange(
                                                  "c (h w) -> c h w", h=83)[
                                                  :, st * 5 + dy : st * 5 + dy + 5,
                                                  dx : dx + 80]
                                              nc.tensor.matmul(
                                                  ps[:, :], wsl4[:, si : si + 64], rhs,
                                                  start=(nmm == 0), stop=(nmm == 15))
                                              nmm += 1
                                      nc.vector.tensor_copy(
                                          raw4[:, 400 * st : 400 * (st + 1)], ps[:, :])
                                  pad5 = l4o.tile([64, 83 * 83], f32)
                                  nc.vector.memset(pad5[:, :], 0.0)
                                  pv5 = pad5[:, :].rearrange("c (h w) -> c h w", h=83)[
                                      :, 1:81, 1:81]
                                  bn_relu(raw4[:, :], 6400, 64, g_sb[:64, 7:8],
                                          be_sb[:64, 7:8], pv5, "4")

                                  if _lvl >= 5:
                                    # ---- L5: 64x80x80 conv 64->1 + tanh -> c ----
                                    with (
                                        tc.tile_pool(name="l5_w", bufs=1) as l5w,
                                        tc.tile_pool(name="l5_out", bufs=1) as l5o,
                                    ):
                                        wsl5 = l5w.tile([64, 16 * 32], f32)
                                        for dy in range(4):
                                            for dx in range(4):
                                                _p5 = (dy * 4 + dx) * 32
                                                nc.sync.dma_start(
                                                    out=wsl5[:, _p5 : _p5 + 32],
                                                    in_=w5t[dy, dx, :, :])
                                        for st in range(16):
                                            ps = bps.tile([32, 400], f32, tag="cps", bufs=2)
                                            nmm = 0
                                            for dy in range(4):
                                                for dx in range(4):
                                                    rhs = pad5[:, :].rearrange(
                                                        "c (h w) -> c h w", h=83)[
                                                        :, st * 5 + dy : st * 5 + dy + 5,
                                                        dx : dx + 80]
                                                    _p5 = (dy * 4 + dx) * 32
                                                    nc.tensor.matmul(
                                                        ps[:, :],
                                                        wsl5[:, _p5 : _p5 + 32],
                                                        rhs,
                                                        start=(nmm == 0), stop=(nmm == 15))
                                                    nmm += 1
                                            c32 = l5o.tile([32, 400], f32, tag="c32", name=f"c32_{st}")
                                            nc.scalar.activation(c32[:, :], ps[:, :], AF.Tanh)
                                            nc.sync.dma_start(
                                                out=c_scr[:, 400 * st : 400 * (st + 1)], in_=c32[:, :])

        # ================= Phase C: w = W_d2 @ c + b_d2 (sharded) ==========
        _skip_c = False
        if not _skip_c:
          with (
              tc.tile_pool(name="c_const", bufs=1) as ccp,
              tc.tile_pool(name="c_slab", bufs=2) as csp,
              tc.tile_pool(name="c_ps", bufs=1, space="PSUM") as cps,
          ):
              c_cols = ccp.tile([128, 50], f32)
              nc.sync.dma_start(
                  out=c_cols[:, :], in_=c_scr[0, :].rearrange("(f p) -> p f", p=128))
              bdc = ccp.tile([128, 5], f32)
              nc.sync.dma_start(out=bdc[:, :], in_=bd2_c[:, :])
              wtiles = {}
              for j in range(5):
                  wt_ps = cps.tile([128, 1], f32, tag=f"wps{j}", name=f"wps{j}")
                  wtiles[j] = wt_ps
              for k in range(50):
                  slab = csp.tile([128, MROWS_C], f32, tag="cslab")
                  nc.sync.dma_start(
                      out=slab[:, :], in_=wd2_t[128 * k : 128 * (k + 1), :])
                  for j in range(5):
                      cj = 128 if j < 4 else 84
                      nc.tensor.matmul(
                          wtiles[j][:cj, :], slab[:, 128 * j : 128 * j + cj],
                          c_cols[:, k : k + 1], start=(k == 0), stop=(k == 49))
              wdc = ccp.tile([128, 5], f32)
              for j in range(5):
                  cj = 128 if j < 4 else 84
                  nc.vector.tensor_tensor(
                      out=wdc[:cj, j : j + 1], in0=wtiles[j][:cj, :],
                      in1=bdc[:cj, j : j + 1], op=OP.add)
              for j in range(5):
                  cj = 128 if j < 4 else 84
                  nc.sync.dma_start(
                      out=wd_shard[128 * j : 128 * j + cj], in_=wdc[:cj, j])
        if not _skip_c:
            nc.gpsimd.collective_compute(
                "AllGather", OP.bypass, replica_groups=[list(range(NCORES))],
                ins=[wd_shard[:]], outs=[w_full[:]])

        if not with_scan:
            with tc.tile_pool(name="wout", bufs=1) as wop:
                w_sb0 = wop.tile([N, N], f32)
                nc.sync.dma_start(
                    out=w_sb0[:, :],
                    in_=w_full[0 : N * N].rearrange("(j i) -> j i", i=N))
                nc.sync.dma_start(out=w_out[:, :], in_=w_sb0[:, :])

        # ================= Phase D: spiking scan (z-space) ================
        # z_{t+1} = z_t - tanh(z_t) @ w accumulated in one PSUM bank
        # (z_t = (-1)^t s_t@w); shadow chain ss_{t+1} = ss_t - v_t on DVE
        # (ss_t = (-1)^t s_t); out[r] = (-1)^(r+1) ss_{r+1}, sign folded
        # into the per-partition scalar of the post-transpose copy.
        if with_scan:
          with (
              tc.tile_pool(name="d_const", bufs=1) as dcp,
              tc.tile_pool(name="d_state", bufs=1) as dsp,
              tc.tile_pool(name="d_blk", bufs=3) as dbp,
              tc.tile_pool(name="d_v", bufs=2) as dvp,
              tc.tile_pool(name="d_ps", bufs=1, space="PSUM") as dps,
              tc.tile_pool(name="t_ps", bufs=2, space="PSUM") as tpp,
              tc.tile_pool(name="t_out", bufs=3) as top,
          ):
              w_sb = dcp.tile([N, N], f32)
              nc.sync.dma_start(
                  out=w_sb[:, :],
                  in_=w_full[0 : N * N].rearrange("(j i) -> j i", i=N))
              negw = dcp.tile([N, N], f32)
              nc.vector.tensor_scalar_mul(negw[:, :], w_sb[:, :], -1.0)
              ident = dcp.tile([128, 128], f32)
              nc.sync.dma_start(out=ident[:, :], in_=ident_in[:, :])
              sgn = dcp.tile([128, 1], f32)
              nc.sync.dma_start(out=sgn[:, :], in_=sgn_in[:, :])
              s0_sb = dsp.tile([N, 1], f32)
              nc.sync.dma_start(out=s0_sb[:, :], in_=s0_in[:, :])
              ns0 = dsp.tile([N, 1], f32)
              nc.vector.tensor_scalar_mul(ns0[:, :], s0_sb[:, :], -1.0)
              s_col = dsp.tile([N, 1], f32)
              nc.vector.tensor_copy(s_col[:, :], s0_sb[:, :])
              ps = dps.tile([N, 1], f32)
              nc.tensor.matmul(ps[:, :], negw[:, :], ns0[:, :], start=True,
                               stop=True)

              n_full = SCAN_B // 128          # full 128-row chunks per block
              tail_w = SCAN_B - n_full * 128  # remaining rows

              def scan_block(off_expr):
                  """B steps; off_expr = global step offset (must be even)."""
                  sblk = dbp.tile([N, SCAN_B], f32, tag="sblk")
                  stage = top.tile([128, (n_full + 1) * N], f32, tag="stage")
                  for k in range(SCAN_B):
                      v = dvp.tile([N, 1], f32, tag="v")
                      nc.scalar.activation(v[:, :], ps[:, :], AF.Tanh)
                      prev = s_col[:, :] if k == 0 else sblk[:, k - 1 : k]
                      nc.vector.tensor_tensor(
                          out=sblk[:, k : k + 1], in0=prev, in1=v[:, :],
                          op=OP.subtract)
                      nc.tensor.matmul(
                          ps[:, :], negw[:, :], v[:, :], start=False,
                          stop=True, skip_group_check=True)
                      if (k + 1) % 128 == 0 or k == SCAN_B - 1:
                          c = k // 128
                          c0 = c * 128
                          wdt = k + 1 - c0
                          tps = tpp.tile([128, N], f32, tag="tps")
                          nc.tensor.transpose(
                              tps[:wdt, :], sblk[:, c0 : c0 + wdt],
                              ident[:N, :N])
                          nc.vector.tensor_scalar_mul(
                              stage[:wdt, c * N : (c + 1) * N],
                              tps[:wdt, :], sgn[:wdt, 0:1])
                  nc.sync.dma_start(
                      out=out_traj[bass.ds(off_expr, n_full * 128), :]
                      .rearrange("(c p) f -> p c f", p=128),
                      in_=stage[:, 0 : n_full * N]
                      .rearrange("p (c f) -> p c f", f=N))
                  nc.sync.dma_start(
                      out=out_traj[bass.ds(off_expr + n_full * 128, tail_w), :],
                      in_=stage[:tail_w, n_full * N : (n_full + 1) * N])
                  nc.vector.tensor_copy(s_col[:, :], sblk[:, SCAN_B - 1 : SCAN_B])

              blk = SCAN_B * scan_u
              if T // blk > 1:
                  with tc.For_i(
                      0, T, blk,
                      hint_engines=(
                          mybir.EngineType.PE, mybir.EngineType.Activation,
                          mybir.EngineType.DVE),
                  ) as iv:
                      for u in range(scan_u):
                          scan_block(iv + u * SCAN_B)
              else:
                  for u in range(scan_u):
                      scan_block(u * SCAN_B)

    return nc


def _marshal_inputs(inputs):
    """Build the 8 per-core input maps from the full problem inputs."""
    x = np.asarray(inputs["x"], np.float32).reshape(2048)
    win = np.asarray(inputs["W_in"], np.float32)
    b_in = np.asarray(inputs["b_in"], np.float32)
    wd2 = np.asarray(inputs["W_d2"], np.float32)
    bd2 = np.asarray(inputs["b_d2"], np.float32)
    sp = np.asarray(inputs["start_part"], np.float32)

    x_cols = np.ascontiguousarray(x.reshape(16, 128).T)
    g_all = np.zeros((128, 8), np.float32)
    be_all = np.zeros((128, 8), np.float32)
    g_all[:, 0:4] = _col_major_pad(np.asarray(inputs["g1"], np.float32), 4)
    g_all[:, 4:6] = _col_major_pad(np.asarray(inputs["g2"], np.float32), 2)
    g_all[:, 6:7] = _col_major_pad(np.asarray(inputs["g3"], np.float32), 1)
    g_all[:, 7:8] = _col_major_pad(np.asarray(inputs["g4"], np.float32), 1)
    be_all[:, 0:4] = _col_major_pad(np.asarray(inputs["be1"], np.float32), 4)
    be_all[:, 4:6] = _col_major_pad(np.asarray(inputs["be2"], np.float32), 2)
    be_all[:, 6:7] = _col_major_pad(np.asarray(inputs["be3"], np.float32), 1)
    be_all[:, 7:8] = _col_major_pad(np.asarray(inputs["be4"], np.float32), 1)
    w1t = np.asarray(inputs["w1"], np.float32).transpose(2, 3, 1, 0)
    w2t = np.asarray(inputs["w2"], np.float32).transpose(2, 3, 1, 0)
    g1 = np.asarray(inputs["g1"], np.float32)
    be1 = np.asarray(inputs["be1"], np.float32)
    g2 = np.asarray(inputs["g2"], np.float32)
    be2 = np.asarray(inputs["be2"], np.float32)
    wts = {
        "w3t": np.ascontiguousarray(
            np.asarray(inputs["w3"], np.float32).transpose(2, 3, 1, 0)),
        "w4t": np.ascontiguousarray(
            np.asarray(inputs["w4"], np.float32).transpose(2, 3, 1, 0)),
        "w5t": _pad_w5(np.asarray(inputs["w5"], np.float32)),
    }
    s0 = np.ascontiguousarray(sp[-1].reshape(N, 1))
    ident = np.eye(128, dtype=np.float32)
    sgn = np.where(np.arange(128) % 2 == 0, -1.0, 1.0).astype(
        np.float32).reshape(128, 1)

    wd2_pad = np.zeros((NCORES * MROWS_C, 6400), np.float32)
    wd2_pad[: wd2.shape[0]] = wd2
    bd2_pad = np.zeros(NCORES * MROWS_C, np.float32)
    bd2_pad[: bd2.shape[0]] = bd2

    in_maps = []
    for c in range(NCORES):
        m = {
            "x_cols": x_cols,
            "win_t": np.ascontiguousarray(
                win[MROWS_A * c : MROWS_A * (c + 1)].T),
            "bin_c": _col_major_pad(b_in[MROWS_A * c : MROWS_A * (c + 1)], 13),
            "g_all": g_all,
            "be_all": be_all,
            "wd2_t": np.ascontiguousarray(
                wd2_pad[MROWS_C * c : MROWS_C * (c + 1)].T),
            "bd2_c": _col_major_pad(bd2_pad[MROWS_C * c : MROWS_C * (c + 1)], 5),
            "s0": s0,
            "ident": ident,
            "sgn": sgn,
            "w1c": np.ascontiguousarray(w1t[:, :, :, 64 * c : 64 * (c + 1)]),
            "w2c": np.ascontiguousarray(w2t[:, :, :, 32 * c : 32 * (c + 1)]),
            "g1c": np.ascontiguousarray(g1[64 * c : 64 * (c + 1)].reshape(64, 1)),
            "be1c": np.ascontiguousarray(be1[64 * c : 64 * (c + 1)].reshape(64, 1)),
            "g2c": np.ascontiguousarray(g2[32 * c : 32 * (c + 1)].reshape(32, 1)),
            "be2c": np.ascontiguousarray(be2[32 * c : 32 * (c + 1)].reshape(32, 1)),
        }
        m.update(wts)
        in_maps.append(m)
    return in_maps


LAST_EXEC_NS = None


def kernel(**inputs) -> np.ndarray:
    global LAST_EXEC_NS
    import os

    trace = bool(os.environ.get("KERNEL_TRACE"))
    nc = build_program(T_FULL)
    _split_excess_waits(nc)
    in_maps = _marshal_inputs(inputs)
    res = run_bass_kernel_spmd(nc, in_maps, list(range(NCORES)), trace=trace)
    if res.exec_time_ns is not None:
        LAST_EXEC_NS = res.exec_time_ns
    out = np.asarray(res.results[0]["out"], np.float32)
    return out.reshape(1, T_FULL, N)


if __name__ == "__main__":
    # CoreSim selftest with a short scan (no hardware needed).
    import sys
    import time

    T_test = SCAN_B * 2
    nc = build_program(T_test)
    print("program built", flush=True)

    sys.path.insert(0, "/root/problem")
    import jax
    jax.config.update("jax_platform_name", "cpu")
    import reference

    inputs = reference.setup_inputs()
    inputs = {k: np.asarray(v) for k, v in inputs.items()}
    in_maps = _marshal_inputs(inputs)

    from concourse.bass_interp import MultiCoreSim

    t0 = time.time()
    sim = MultiCoreSim(nc, NCORES)
    for i in range(NCORES):
        for k, v in in_maps[i].items():
            sim.cores[i].tensor(k)[:] = v
    sim.simulate()
    print("sim time", time.time() - t0, flush=True)
    got = np.array(sim.cores[0].tensor("out"))

    # host reference for the short horizon
    w = np.load("/tmp/w.npy")
    s = np.asarray(inputs["start_part"])[-1].astype(np.float32)
    ref = np.empty((T_test, N), np.float32)
    for t in range(T_test):
        s = (np.tanh((s @ w).astype(np.float32)).astype(np.float32) - s).astype(
            np.float32)
        ref[t] = s
    err = np.abs(got - ref)
    rel = np.abs(got - ref) / (np.abs(ref) + 1e-6)
    print("traj absmax err:", err.max(), "rel max:", rel.max())
    print("first rows got:", got[0, :4], "ref:", ref[0, :4])



# revision 21
# speedup vs baseline: 1.0017x; 1.0017x over previous
"""Trainium2 Bass kernel for nn_DCGAN_G (DCGAN generator + 69-neuron spiking scan).

Strategy (8 NeuronCores, SPMD):
  A. W_in matvec (12800x2048) row-sharded 8x -> AllGather h1 (12800).
  B. DCGAN conv stack: L1 (512ch) and L2 (256ch) output-channel-sharded
     8x (per-core weight/bn-param slices keep the program SPMD; bn batch
     stats are per-channel so they stay core-local) with small
     AllGathers of the activations; L3-L5 replicated (<=128 out
     channels, where the PE cost model charges per moving column and
     channel sharding would not reduce cost).
  C. W_d2 matvec (4761x6400) row-sharded 8x -> AllGather w (69x69).
  D. 99800-step spiking recurrence (inherently serial), restructured to
     a 2-engine critical cycle in "z-space": with z_t = (-1)^t s_t @ w,
     the step  s_{t+1} = tanh(s_t@w) - s_t  becomes
         z_{t+1} = z_t - tanh(z_t) @ w
     i.e. one PE matmul accumulating -w^T v into a single persistent
     PSUM bank + one ACT tanh per step (267.7 ns critical cycle:
     173 PE write pipeline + 94 sem hops), vs the naive 3-engine
     {matmul, tanh, subtract} chain at 315 ns. The output states follow
     from the shadow chain ss_{t+1} = ss_t - v_t (ss_t = (-1)^t s_t) on
     the otherwise-idle DVE, off the critical path. (69,B) state blocks
     are PE-transposed to row-major inline (hidden under the scan), the
     (-1)^(r+1) row sign is folded into the per-partition scalar of the
     post-transpose copy, and outputs go to DRAM in 2 strided DMAs per
     998-step block (<=16 dynamic-DMA registers per hardware loop
     iteration; the For_i unrolls 5 blocks = 4990 steps per iteration).
"""
import numpy as np

import bass_rust
import concourse.bass as bass
import concourse.mybir as mybir
from concourse.bass_utils import run_bass_kernel_spmd
from concourse.tile import TileContext
from concourse.vector_clock import ScopedClock

f32 = mybir.dt.float32
AF = mybir.ActivationFunctionType
OP = mybir.AluOpType
AX = mybir.AxisListType

T_FULL = 99800
N = 69
NCORES = 8
EPS = 1e-5
SCAN_B = 998          # even (sign-folding needs even block starts)
SCAN_U = 5            # unrolled blocks per For_i iteration (4990 steps/iter)
MROWS_A = 1600        # W_in rows per core
MROWS_C = 596         # W_d2 rows per core (8*596=4768 >= 4761)


# ---------------------------------------------------------------------------
# walrus workaround: CTRL-type instructions accept at most 1 sem wait, but the
# TileContext tail drain gets one wait per active proc. Split across drains.
def _patched_drain_and_barrier(self, tick_clock, wait_clock):
    drain_inst = self.nc.sync.drain()
    wait_clock.add_sem_waits(
        drain_inst.ins, ScopedClock({None: tick_clock.global_clock})
    )
    si = drain_inst.ins.sync_info
    waits = list(si.on_wait) if si is not None else []
    if len(waits) > 1:
        drain_inst.ins.sync_info = bass_rust.SyncInfo(
            on_wait=waits[:1], on_update=list(si.on_update)
        )
        for i in range(1, len(waits)):
            extra = self.nc.sync.drain()
            extra.ins.sync_info = bass_rust.SyncInfo(
                on_wait=waits[i : i + 1], on_update=[]
            )
    self.nc.all_engine_barrier()
    assert self.sems is not None
    popped = self.nc._tile_sem_poison_stack.pop()
    assert popped is self._sem_poison
    self.nc.clear_and_free_semaphores(list(self.sems.allocated().values()))
    self.nc.all_engine_barrier()


TileContext._drain_and_barrier = _patched_drain_and_barrier
# ---------------------------------------------------------------------------


def _split_excess_waits(nc, max_waits=1):
    """This walrus build accepts at most one sem wait per instruction; move
    excess waits onto single-wait NOPs inserted just before the owner."""
    n_split = 0
    for f in nc.m.functions:
        for b in f.blocks:
            insts = list(b.instructions)
            out = []
            changed = False
            for inst in insts:
                si = inst.sync_info
                waits = list(si.on_wait) if si is not None else []
                if len(waits) > max_waits:
                    changed = True
                    for i, w in enumerate(waits[max_waits:]):
                        nop = mybir.InstNoOp(
                            name=f"wsp_{inst.name}_{i}", ins=[], outs=[])
                        nop.engine = inst.engine
                        nop.sync_info = bass_rust.SyncInfo(
                            on_wait=[w], on_update=[])
                        out.append(nop)
                        n_split += 1
                    inst.sync_info = bass_rust.SyncInfo(
                        on_wait=waits[:max_waits], on_update=list(si.on_update))
                out.append(inst)
            if changed:
                b.instructions = out
    return n_split


def _pad_w5(w5):
    """(1,64,4,4) -> (4,4,64,32) with real weights in out-column 0."""
    t = np.zeros((4, 4, 64, 32), np.float32)
    t[:, :, :, 0:1] = w5.transpose(2, 3, 1, 0)
    return np.ascontiguousarray(t)


def _col_major_pad(v, ncols):
    """(n,) -> (128, ncols) with element m at [m % 128, m // 128], zero pad."""
    out = np.zeros(128 * ncols, np.float32)
    out[: v.shape[0]] = v
    return np.ascontiguousarray(out.reshape(ncols, 128).T)


def build_program(T=T_FULL, with_front=True, with_scan=True):
    nc = bass.Bass()
    scan_u = SCAN_U if T % (SCAN_B * SCAN_U) == 0 else 1
    assert T % (SCAN_B * scan_u) == 0, "T must be a multiple of the block"

    # ---- inputs ----
    x_cols = nc.declare_dram_parameter("x_cols", [128, 16], f32, isOutput=False)
    win_t = nc.declare_dram_parameter("win_t", [2048, MROWS_A], f32, isOutput=False)
    bin_c = nc.declare_dram_parameter("bin_c", [128, 13], f32, isOutput=False)
    w1c = nc.declare_dram_parameter("w1c", [4, 4, 512, 64], f32, isOutput=False)
    w2c = nc.declare_dram_parameter("w2c", [4, 4, 512, 32], f32, isOutput=False)
    g1c_in = nc.declare_dram_parameter("g1c", [64, 1], f32, isOutput=False)
    be1c_in = nc.declare_dram_parameter("be1c", [64, 1], f32, isOutput=False)
    g2c_in = nc.declare_dram_parameter("g2c", [32, 1], f32, isOutput=False)
    be2c_in = nc.declare_dram_parameter("be2c", [32, 1], f32, isOutput=False)
    w3t = nc.declare_dram_parameter("w3t", [4, 4, 256, 128], f32, isOutput=False)
    w4t = nc.declare_dram_parameter("w4t", [4, 4, 128, 64], f32, isOutput=False)
    w5t = nc.declare_dram_parameter("w5t", [4, 4, 64, 32], f32, isOutput=False)
    g_all = nc.declare_dram_parameter("g_all", [128, 8], f32, isOutput=False)
    be_all = nc.declare_dram_parameter("be_all", [128, 8], f32, isOutput=False)
    wd2_t = nc.declare_dram_parameter("wd2_t", [6400, MROWS_C], f32, isOutput=False)
    bd2_c = nc.declare_dram_parameter("bd2_c", [128, 5], f32, isOutput=False)
    s0_in = nc.declare_dram_parameter("s0", [N, 1], f32, isOutput=False)
    ident_in = nc.declare_dram_parameter("ident", [128, 128], f32, isOutput=False)
    sgn_in = nc.declare_dram_parameter("sgn", [128, 1], f32, isOutput=False)
    if with_scan:
        out_traj = nc.declare_dram_parameter("out", [T, N], f32, isOutput=True)
    else:
        w_out = nc.declare_dram_parameter("w_out", [N, N], f32, isOutput=True)

    # ---- internal DRAM ----
    h_shard = nc.dram_tensor("h_shard", [MROWS_A], f32)
    h_full = nc.dram_tensor("h_full", [NCORES * MROWS_A], f32, addr_space="Shared")
    l1_shard = nc.dram_tensor("l1_shard", [64 * 100], f32)
    l1_full = nc.dram_tensor("l1_full", [512 * 100], f32, addr_space="Shared")
    l2_shard = nc.dram_tensor("l2_shard", [32 * 400], f32)
    l2_full = nc.dram_tensor("l2_full", [256 * 400], f32, addr_space="Shared")
    c_scr = nc.dram_tensor("c_scr", [32, 6400], f32)
    wd_shard = nc.dram_tensor("wd_shard", [MROWS_C], f32)
    w_full = nc.dram_tensor("w_full", [NCORES * MROWS_C], f32, addr_space="Shared")

    with TileContext(nc) as tc:
        # ================= Phase A: h = W_in @ x + b_in (sharded) ==========
        with (
            tc.tile_pool(name="a_const", bufs=1) as acp,
            tc.tile_pool(name="a_slab", bufs=2) as asp,
            tc.tile_pool(name="a_ps", bufs=1, space="PSUM") as aps,
        ):
            xc = acp.tile([128, 16], f32)
            nc.sync.dma_start(out=xc[:, :], in_=x_cols[:, :])
            bc = acp.tile([128, 13], f32)
            nc.sync.dma_start(out=bc[:, :], in_=bin_c[:, :])
            hc = acp.tile([128, 13], f32)
            for jlo, jhi in ((0, 8), (8, 13)):
                ptiles = {}
                for j in range(jlo, jhi):
                    pt = aps.tile([128, 1], f32, tag=f"hps{j - jlo}", name=f"hps{j}")
                    ptiles[j] = pt
                for k in range(16):
                    gw = min(128 * jhi, MROWS_A) - 128 * jlo
                    slab = asp.tile([128, 1024], f32, tag="aslab")
                    nc.sync.dma_start(
                        out=slab[:, :gw],
                        in_=win_t[128 * k : 128 * (k + 1),
                                  128 * jlo : 128 * jlo + gw])
                    for j in range(jlo, jhi):
                        cj = 128 if j < 12 else 64
                        jj = j - jlo
                        nc.tensor.matmul(
                            ptiles[j][:cj, :],
                            slab[:, 128 * jj : 128 * jj + cj],
                            xc[:, k : k + 1],
                            start=(k == 0),
                            stop=(k == 15),
                        )
                for j in range(jlo, jhi):
                    cj = 128 if j < 12 else 64
                    nc.vector.tensor_tensor(
                        out=hc[:cj, j : j + 1], in0=ptiles[j][:cj, :],
                        in1=bc[:cj, j : j + 1], op=OP.add)
            for j in range(13):
                cj = 128 if j < 12 else 64
                nc.sync.dma_start(
                    out=h_shard[128 * j : 128 * j + cj], in_=hc[:cj, j])
        nc.gpsimd.collective_compute(
            "AllGather", OP.bypass, replica_groups=[list(range(NCORES))],
            ins=[h_shard[:]], outs=[h_full[:]])

        # ====== Phase B: conv stack (L1/L2 channel-sharded, rest replicated)
        _lvl = 9  # all conv layers (bisection gates left in place, fully on)
        h2d = h_full.rearrange("(c hw) -> c hw", hw=25)

        with (
            tc.tile_pool(name="bn_const", bufs=1) as bnp,
            tc.tile_pool(name="conv_ps", bufs=1, space="PSUM") as bps,
        ):
            g_sb = bnp.tile([128, 8], f32)
            nc.sync.dma_start(out=g_sb[:, :], in_=g_all[:, :])
            be_sb = bnp.tile([128, 8], f32)
            nc.sync.dma_start(out=be_sb[:, :], in_=be_all[:, :])
            g1c_sb = bnp.tile([64, 1], f32)
            nc.sync.dma_start(out=g1c_sb[:, :], in_=g1c_in[:, :])
            be1c_sb = bnp.tile([64, 1], f32)
            nc.sync.dma_start(out=be1c_sb[:, :], in_=be1c_in[:, :])
            g2c_sb = bnp.tile([32, 1], f32)
            nc.sync.dma_start(out=g2c_sb[:, :], in_=g2c_in[:, :])
            be2c_sb = bnp.tile([32, 1], f32)
            nc.sync.dma_start(out=be2c_sb[:, :], in_=be2c_in[:, :])

            def bn_relu(raw, hw, cch, g_ap, be_ap, out_ap, name):
                """BatchNorm(train) + ReLU from raw (cch,hw) into out_ap."""
                with tc.tile_pool(name=f"bn{name}", bufs=1) as p:
                    s1 = p.tile([cch, 1], f32, tag="s1")
                    nc.vector.tensor_reduce(s1[:, :], raw, axis=AX.X, op=OP.add)
                    mean = p.tile([cch, 1], f32, tag="mean")
                    nc.vector.tensor_scalar_mul(mean[:, :], s1[:, :], 1.0 / hw)
                    sq = p.tile([cch, hw], f32, tag="sq")
                    nc.vector.tensor_tensor(out=sq[:, :], in0=raw, in1=raw, op=OP.mult)
                    s2 = p.tile([cch, 1], f32, tag="s2")
                    nc.vector.tensor_reduce(s2[:, :], sq[:, :], axis=AX.X, op=OP.add)
                    ex2 = p.tile([cch, 1], f32, tag="ex2")
                    nc.vector.tensor_scalar_mul(ex2[:, :], s2[:, :], 1.0 / hw)
                    msq = p.tile([cch, 1], f32, tag="msq")
                    nc.vector.tensor_tensor(
                        out=msq[:, :], in0=mean[:, :], in1=mean[:, :], op=OP.mult)
                    var = p.tile([cch, 1], f32, tag="var")
                    nc.vector.tensor_tensor(
                        out=var[:, :], in0=ex2[:, :], in1=msq[:, :], op=OP.subtract)
                    vps = p.tile([cch, 1], f32, tag="vps")
                    nc.vector.tensor_scalar_add(vps[:, :], var[:, :], EPS)
                    sd = p.tile([cch, 1], f32, tag="sd")
                    nc.scalar.activation(sd[:, :], vps[:, :], AF.Sqrt)
                    rstd = p.tile([cch, 1], f32, tag="rstd")
                    nc.vector.reciprocal(rstd[:, :], sd[:, :])
                    scale = p.tile([cch, 1], f32, tag="scale")
                    nc.vector.tensor_tensor(
                        out=scale[:, :], in0=g_ap, in1=rstd[:, :], op=OP.mult)
                    t1 = p.tile([cch, 1], f32, tag="t1")
                    nc.vector.tensor_tensor(
                        out=t1[:, :], in0=mean[:, :], in1=scale[:, :], op=OP.mult)
                    bia = p.tile([cch, 1], f32, tag="bia")
                    nc.vector.tensor_tensor(
                        out=bia[:, :], in0=be_ap, in1=t1[:, :], op=OP.subtract)
                    nc.scalar.activation(
                        out_ap, raw, AF.Relu, bias=bia[:, :], scale=scale[:, :])

            # ---- L1: up2(h:512x5x5)->512x10x10 conv 512->512 ----
            with (
                tc.tile_pool(name="l1_in", bufs=1) as l1i,
                tc.tile_pool(name="l1_w", bufs=2) as l1w,
                tc.tile_pool(name="l1_out", bufs=1) as l1o,
            ):
                pads1 = []
                for j in range(4):
                    hm = l1i.tile([128, 25], f32, tag=f"hm{j}")
                    nc.sync.dma_start(out=hm[:, :], in_=h2d[128 * j : 128 * (j + 1), :])
                    pad = l1i.tile([128, 13 * 13], f32, tag=f"pad1_{j}")
                    nc.vector.memset(pad[:, :], 0.0)
                    pv = pad[:, :].rearrange("c (h w) -> c h w", h=13)
                    hv = hm[:, :].rearrange("c (h w) -> c h w", h=5)
                    for a in range(2):
                        for b in range(2):
                            nc.vector.tensor_copy(
                                pv[:, a + 1 : a + 11 : 2, b + 1 : b + 11 : 2], hv[:, :, :])
                    pads1.append(pad)
                ps1 = bps.tile([64, 100], f32, tag="l1ps", name="l1ps")
                nmm = 0
                for ji in range(4):
                    for dy in range(4):
                        for dx in range(4):
                            slab = l1w.tile([128, 64], f32, tag="w1slab")
                            nc.sync.dma_start(
                                out=slab[:, :],
                                in_=w1c[dy, dx, 128 * ji : 128 * (ji + 1), :])
                            rhs = pads1[ji][:, :].rearrange(
                                "c (h w) -> c h w", h=13)[:, dy : dy + 10, dx : dx + 10]
                            nc.tensor.matmul(
                                ps1[:, :], slab[:, :], rhs,
                                start=(nmm == 0), stop=(nmm == 63))
                            nmm += 1
                raw = l1o.tile([64, 100], f32, tag="raw1")
                nc.vector.tensor_copy(raw[:, :], ps1[:, :])
                relu = l1o.tile([64, 100], f32, tag="relu1")
                bn_relu(raw[:, :], 100, 64, g1c_sb[:, :], be1c_sb[:, :],
                        relu[:, :], "1")
                nc.sync.dma_start(
                    out=l1_shard[:].rearrange("(c hw) -> c hw", hw=100),
                    in_=relu[:, :])
                nc.gpsimd.collective_compute(
                    "AllGather", OP.bypass,
                    replica_groups=[list(range(NCORES))],
                    ins=[l1_shard[:]], outs=[l1_full[:]])
                l1_2d = l1_full.rearrange("(c hw) -> c hw", hw=100)
                pads2 = []
                for jo in range(4):
                    rm = l1o.tile([128, 100], f32, tag=f"rm1_{jo}")
                    nc.sync.dma_start(
                        out=rm[:, :], in_=l1_2d[128 * jo : 128 * (jo + 1), :])
                    pad = l1o.tile([128, 23 * 23], f32, tag=f"pad2_{jo}")
                    nc.vector.memset(pad[:, :], 0.0)
                    pv = pad[:, :].rearrange("c (h w) -> c h w", h=23)
                    rv = rm[:, :].rearrange("c (h w) -> c h w", h=10)
                    for a in range(2):
                        for b in range(2):
                            nc.vector.tensor_copy(
                                pv[:, a + 1 : a + 21 : 2, b + 1 : b + 21 : 2], rv[:, :, :])
                    pads2.append(pad)

                if _lvl >= 2:
                  # ---- L2: 512x20x20 conv 512->256 ----
                  with (
                      tc.tile_pool(name="l2_w", bufs=2) as l2w,
                      tc.tile_pool(name="l2_out", bufs=1) as l2o,
                  ):
                      psA = bps.tile([32, 400], f32, tag="cpsA")
                      nmm = 0
                      for ji in range(4):
                          for dy in range(4):
                              for dx in range(4):
                                  slab = l2w.tile([128, 32], f32, tag="w2slab")
                                  nc.sync.dma_start(
                                      out=slab[:, :],
                                      in_=w2c[dy, dx, 128 * ji : 128 * (ji + 1), :])
                                  rhs = pads2[ji][:, :].rearrange(
                                      "c (h w) -> c h w", h=23)[:, dy : dy + 20, dx : dx + 20]
                                  nc.tensor.matmul(
                                      psA[:, :], slab[:, :], rhs,
                                      start=(nmm == 0), stop=(nmm == 63))
                                  nmm += 1
                      raw2 = l2o.tile([32, 400], f32, tag="raw2")
                      nc.vector.tensor_copy(raw2[:, :], psA[:, :])
                      relu2 = l2o.tile([32, 400], f32, tag="relu2")
                      bn_relu(raw2[:, :], 400, 32, g2c_sb[:, :], be2c_sb[:, :],
                              relu2[:, :], "2")
                      nc.sync.dma_start(
                          out=l2_shard[:].rearrange("(c hw) -> c hw", hw=400),
                          in_=relu2[:, :])
                      nc.gpsimd.collective_compute(
                          "AllGather", OP.bypass,
                          replica_groups=[list(range(NCORES))],
                          ins=[l2_shard[:]], outs=[l2_full[:]])
                      l2_2d = l2_full.rearrange("(c hw) -> c hw", hw=400)
                      pads3 = []
                      for jo in range(2):
                          rm2 = l2o.tile([128, 400], f32, tag=f"rm2_{jo}")
                          nc.sync.dma_start(
                              out=rm2[:, :],
                              in_=l2_2d[128 * jo : 128 * (jo + 1), :])
                          pad = l2o.tile([128, 43 * 43], f32, tag=f"pad3_{jo}")
                          nc.vector.memset(pad[:, :], 0.0)
                          pv = pad[:, :].rearrange("c (h w) -> c h w", h=43)
                          rv = rm2[:, :].rearrange("c (h w) -> c h w", h=20)
                          for a in range(2):
                              for b in range(2):
                                  nc.vector.tensor_copy(
                                      pv[:, a + 1 : a + 41 : 2, b + 1 : b + 41 : 2],
                                      rv[:, :, :])
                          pads3.append(pad)

                      if _lvl >= 3:
                        # ---- L3: 256x40x40 conv 256->128 ----
                        with (
                            tc.tile_pool(name="l3_w", bufs=1) as l3w,
                            tc.tile_pool(name="l3_out", bufs=1) as l3o,
                        ):
                            wsl3 = l3w.tile([128, 32 * 128], f32)
                            for ji in range(2):
                                for dy in range(4):
                                    for dx in range(4):
                                        si = (ji * 16 + dy * 4 + dx) * 128
                                        nc.sync.dma_start(
                                            out=wsl3[:, si : si + 128],
                                            in_=w3t[dy, dx, 128 * ji : 128 * (ji + 1), :])
                            raw3 = l3o.tile([128, 1600], f32)
                            for st in range(4):
                                ps = bps.tile([128, 400], f32, tag="cps", bufs=2)
                                nmm = 0
                                for ji in range(2):
                                    for dy in range(4):
                                        for dx in range(4):
                                            si = (ji * 16 + dy * 4 + dx) * 128
                                            rhs = pads3[ji][:, :].rearrange(
                                                "c (h w) -> c h w", h=43)[
                                                :, st * 10 + dy : st * 10 + dy + 10,
                                                dx : dx + 40]
                                            nc.tensor.matmul(
                                                ps[:, :], wsl3[:, si : si + 128], rhs,
                                                start=(nmm == 0), stop=(nmm == 31))
                                            nmm += 1
                                nc.vector.tensor_copy(
                                    raw3[:, 400 * st : 400 * (st + 1)], ps[:, :])
                            relu3 = l3o.tile([128, 1600], f32)
                            bn_relu(raw3[:, :], 1600, 128, g_sb[:128, 6:7],
                                    be_sb[:128, 6:7], relu3[:, :], "3")
                            pad4 = l3o.tile([128, 83 * 83], f32)
                            nc.vector.memset(pad4[:, :], 0.0)
                            pv = pad4[:, :].rearrange("c (h w) -> c h w", h=83)
                            rv = relu3[:, :].rearrange("c (h w) -> c h w", h=40)
                            for a in range(2):
                                for b in range(2):
                                    nc.vector.tensor_copy(
                                        pv[:, a + 1 : a + 81 : 2, b + 1 : b + 81 : 2],
                                        rv[:, :, :])

                            if _lvl >= 4:
                              # ---- L4: 128x80x80 conv 128->64 ----
                              with (
                                  tc.tile_pool(name="l4_w", bufs=1) as l4w,
                                  tc.tile_pool(name="l4_out", bufs=1) as l4o,
                              ):
                                  wsl4 = l4w.tile([128, 16 * 64], f32)
                                  for dy in range(4):
                                      for dx in range(4):
                                          si = (dy * 4 + dx) * 64
                                          nc.sync.dma_start(
                                              out=wsl4[:, si : si + 64],
                                              in_=w4t[dy, dx, :, :])
                                  raw4 = l4o.tile([64, 6400], f32)
                                  for st in range(16):
                                      ps = bps.tile([64, 400], f32, tag="cps", bufs=2)
                                      nmm = 0
                                      for dy in range(4):
                                          for dx in range(4):
                                              si = (dy * 4 + dx) * 64
                                              rhs = pad4[:, :].rearrange(
                                                  "c (h w) -> c h w", h=83)[
                                                  :, st * 5 + dy : st * 5 + dy + 5,
                                                  dx : dx + 80]
                                              nc.tensor.matmul(
                                                  ps[:, :], wsl4[:, si : si + 64], rhs,
                                                  start=(nmm == 0), stop=(nmm == 15))
                                              nmm += 1
                                      nc.vector.tensor_copy(
                                          raw4[:, 400 * st : 400 * (st + 1)], ps[:, :])
                                  pad5 = l4o.tile([64, 83 * 83], f32)
                                  nc.vector.memset(pad5[:, :], 0.0)
                                  pv5 = pad5[:, :].rearrange("c (h w) -> c h w", h=83)[
                                      :, 1:81, 1:81]
                                  bn_relu(raw4[:, :], 6400, 64, g_sb[:64, 7:8],
                                          be_sb[:64, 7:8], pv5, "4")

                                  if _lvl >= 5:
                                    # ---- L5: 64x80x80 conv 64->1 + tanh -> c ----
                                    with (
                                        tc.tile_pool(name="l5_w", bufs=1) as l5w,
                                        tc.tile_pool(name="l5_out", bufs=1) as l5o,
                                    ):
                                        wsl5 = l5w.tile([64, 16 * 32], f32)
                                        for dy in range(4):
                                            for dx in range(4):
                                                _p5 = (dy * 4 + dx) * 32
                                                nc.sync.dma_start(
                                                    out=wsl5[:, _p5 : _p5 + 32],
                                                    in_=w5t[dy, dx, :, :])
                                        for st in range(16):
                                            ps = bps.tile([32, 400], f32, tag="cps", bufs=2)
                                            nmm = 0
                                            for dy in range(4):
                                                for dx in range(4):
                                                    rhs = pad5[:, :].rearrange(
                                                        "c (h w) -> c h w", h=83)[
                                                        :, st * 5 + dy : st * 5 + dy + 5,
                                                        dx : dx + 80]
                                                    _p5 = (dy * 4 + dx) * 32
                                                    nc.tensor.matmul(
                                                        ps[:, :],
                                                        wsl5[:, _p5 : _p5 + 32],
                                                        rhs,
                                                        start=(nmm == 0), stop=(nmm == 15))
                                                    nmm += 1
                                            c32 = l5o.tile([32, 400], f32, tag="c32", name=f"c32_{st}")
                                            nc.scalar.activation(c32[:, :], ps[:, :], AF.Tanh)
                                            nc.sync.dma_start(
                                                out=c_scr[:, 400 * st : 400 * (st + 1)], in_=c32[:, :])

        # ================= Phase C: w = W_d2 @ c + b_d2 (sharded) ==========
        _skip_c = False
        if not _skip_c:
          with (
              tc.tile_pool(name="c_const", bufs=1) as ccp,
              tc.tile_pool(name="c_slab", bufs=2) as csp,
              tc.tile_pool(name="c_ps", bufs=1, space="PSUM") as cps,
          ):
              c_cols = ccp.tile([128, 50], f32)
              nc.sync.dma_start(
                  out=c_cols[:, :], in_=c_scr[0, :].rearrange("(f p) -> p f", p=128))
              bdc = ccp.tile([128, 5], f32)
              nc.sync.dma_start(out=bdc[:, :], in_=bd2_c[:, :])
              wtiles = {}
              for j in range(5):
                  wt_ps = cps.tile([128, 1], f32, tag=f"wps{j}", name=f"wps{j}")
                  wtiles[j] = wt_ps
              for k in range(50):
                  slab = csp.tile([128, MROWS_C], f32, tag="cslab")
                  nc.sync.dma_start(
                      out=slab[:, :], in_=wd2_t[128 * k : 128 * (k + 1), :])
                  for j in range(5):
                      cj = 128 if j < 4 else 84
                      nc.tensor.matmul(
                          wtiles[j][:cj, :], slab[:, 128 * j : 128 * j + cj],
                          c_cols[:, k : k + 1], start=(k == 0), stop=(k == 49))
              wdc = ccp.tile([128, 5], f32)
              for j in range(5):
                  cj = 128 if j < 4 else 84
                  nc.vector.tensor_tensor(
                      out=wdc[:cj, j : j + 1], in0=wtiles[j][:cj, :],
                      in1=bdc[:cj, j : j + 1], op=OP.add)
              for j in range(5):
                  cj = 128 if j < 4 else 84
                  nc.sync.dma_start(
                      out=wd_shard[128 * j : 128 * j + cj], in_=wdc[:cj, j])
        if not _skip_c:
            nc.gpsimd.collective_compute(
                "AllGather", OP.bypass, replica_groups=[list(range(NCORES))],
                ins=[wd_shard[:]], outs=[w_full[:]])

        if not with_scan:
            with tc.tile_pool(name="wout", bufs=1) as wop:
                w_sb0 = wop.tile([N, N], f32)
                nc.sync.dma_start(
                    out=w_sb0[:, :],
                    in_=w_full[0 : N * N].rearrange("(j i) -> j i", i=N))
                nc.sync.dma_start(out=w_out[:, :], in_=w_sb0[:, :])

        # ================= Phase D: spiking scan (z-space) ================
        # z_{t+1} = z_t - tanh(z_t) @ w accumulated in one PSUM bank
        # (z_t = (-1)^t s_t@w); shadow chain ss_{t+1} = ss_t - v_t on DVE
        # (ss_t = (-1)^t s_t); out[r] = (-1)^(r+1) ss_{r+1}, sign folded
        # into the per-partition scalar of the post-transpose copy.
        if with_scan:
          with (
              tc.tile_pool(name="d_const", bufs=1) as dcp,
              tc.tile_pool(name="d_state", bufs=1) as dsp,
              tc.tile_pool(name="d_blk", bufs=3) as dbp,
              tc.tile_pool(name="d_v", bufs=2) as dvp,
              tc.tile_pool(name="d_ps", bufs=1, space="PSUM") as dps,
              tc.tile_pool(name="t_ps", bufs=2, space="PSUM") as tpp,
              tc.tile_pool(name="t_out", bufs=3) as top,
          ):
              w_sb = dcp.tile([N, N], f32)
              nc.sync.dma_start(
                  out=w_sb[:, :],
                  in_=w_full[0 : N * N].rearrange("(j i) -> j i", i=N))
              negw = dcp.tile([N, N], f32)
              nc.vector.tensor_scalar_mul(negw[:, :], w_sb[:, :], -1.0)
              ident = dcp.tile([128, 128], f32)
              nc.sync.dma_start(out=ident[:, :], in_=ident_in[:, :])
              sgn = dcp.tile([128, 1], f32)
              nc.sync.dma_start(out=sgn[:, :], in_=sgn_in[:, :])
              s0_sb = dsp.tile([N, 1], f32)
              nc.sync.dma_start(out=s0_sb[:, :], in_=s0_in[:, :])
              ns0 = dsp.tile([N, 1], f32)
              nc.vector.tensor_scalar_mul(ns0[:, :], s0_sb[:, :], -1.0)
              s_col = dsp.tile([N, 1], f32)
              nc.vector.tensor_copy(s_col[:, :], s0_sb[:, :])
              ps = dps.tile([N, 1], f32)
              nc.tensor.matmul(ps[:, :], negw[:, :], ns0[:, :], start=True,
                               stop=True)

              n_full = SCAN_B // 128          # full 128-row chunks per block
              tail_w = SCAN_B - n_full * 128  # remaining rows

              def scan_block(off_expr):
                  """B steps; off_expr = global step offset (must be even)."""
                  sblk = dbp.tile([N, SCAN_B], f32, tag="sblk")
                  stage = top.tile([128, (n_full + 1) * N], f32, tag="stage")
                  for k in range(SCAN_B):
                      v = dvp.tile([N, 1], f32, tag="v")
                      nc.scalar.activation(v[:, :], ps[:, :], AF.Tanh)
                      prev = s_col[:, :] if k == 0 else sblk[:, k - 1 : k]
                      nc.vector.tensor_tensor(
                          out=sblk[:, k : k + 1], in0=prev, in1=v[:, :],
                          op=OP.subtract)
                      nc.tensor.matmul(
                          ps[:, :], negw[:, :], v[:, :], start=False,
                          stop=True, skip_group_check=True)
                      if (k + 1) % 128 == 0 or k == SCAN_B - 1:
                          c = k // 128
                          c0 = c * 128
                          wdt = k + 1 - c0
                          tps = tpp.tile([128, N], f32, tag="tps")
                          nc.tensor.transpose(
                              tps[:wdt, :], sblk[:, c0 : c0 + wdt],
                              ident[:N, :N])
                          nc.vector.tensor_scalar_mul(
                              stage[:wdt, c * N : (c + 1) * N],
                              tps[:wdt, :], sgn[:wdt, 0:1])
                  nc.sync.dma_start(
                      out=out_traj[bass.ds(off_expr, n_full * 128), :]
                      .rearrange("(c p) f -> p c f", p=128),
                      in_=stage[:, 0 : n_full * N]
                      .rearrange("p (c f) -> p c f", f=N))
                  nc.sync.dma_start(
                      out=out_traj[bass.ds(off_expr + n_full * 128, tail_w), :],
                      in_=stage[:tail_w, n_full * N : (n_full + 1) * N])
                  nc.vector.tensor_copy(s_col[:, :], sblk[:, SCAN_B - 1 : SCAN_B])

              blk = SCAN_B * scan_u
              if T // blk > 1:
                  with tc.For_i(
                      0, T, blk,
                      hint_engines=(
                          mybir.EngineType.PE, mybir.EngineType.Activation,
                          mybir.EngineType.DVE),
                  ) as iv:
                      for u in range(scan_u):
                          scan_block(iv + u * SCAN_B)
              else:
                  for u in range(scan_u):
                      scan_block(u * SCAN_B)

    return nc


def _marshal_inputs(inputs):
    """Build the 8 per-core input maps from the full problem inputs."""
    x = np.asarray(inputs["x"], np.float32).reshape(2048)
    win = np.asarray(inputs["W_in"], np.float32)
    b_in = np.asarray(inputs["b_in"], np.float32)
    wd2 = np.asarray(inputs["W_d2"], np.float32)
    bd2 = np.asarray(inputs["b_d2"], np.float32)
    sp = np.asarray(inputs["start_part"], np.float32)

    x_cols = np.ascontiguousarray(x.reshape(16, 128).T)
    g_all = np.zeros((128, 8), np.float32)
    be_all = np.zeros((128, 8), np.float32)
    g_all[:, 0:4] = _col_major_pad(np.asarray(inputs["g1"], np.float32), 4)
    g_all[:, 4:6] = _col_major_pad(np.asarray(inputs["g2"], np.float32), 2)
    g_all[:, 6:7] = _col_major_pad(np.asarray(inputs["g3"], np.float32), 1)
    g_all[:, 7:8] = _col_major_pad(np.asarray(inputs["g4"], np.float32), 1)
    be_all[:, 0:4] = _col_major_pad(np.asarray(inputs["be1"], np.float32), 4)
    be_all[:, 4:6] = _col_major_pad(np.asarray(inputs["be2"], np.float32), 2)
    be_all[:, 6:7] = _col_major_pad(np.asarray(inputs["be3"], np.float32), 1)
    be_all[:, 7:8] = _col_major_pad(np.asarray(inputs["be4"], np.float32), 1)
    w1t = np.asarray(inputs["w1"], np.float32).transpose(2, 3, 1, 0)
    w2t = np.asarray(inputs["w2"], np.float32).transpose(2, 3, 1, 0)
    g1 = np.asarray(inputs["g1"], np.float32)
    be1 = np.asarray(inputs["be1"], np.float32)
    g2 = np.asarray(inputs["g2"], np.float32)
    be2 = np.asarray(inputs["be2"], np.float32)
    wts = {
        "w3t": np.ascontiguousarray(
            np.asarray(inputs["w3"], np.float32).transpose(2, 3, 1, 0)),
        "w4t": np.ascontiguousarray(
            np.asarray(inputs["w4"], np.float32).transpose(2, 3, 1, 0)),
        "w5t": _pad_w5(np.asarray(inputs["w5"], np.float32)),
    }
    s0 = np.ascontiguousarray(sp[-1].reshape(N, 1))
    ident = np.eye(128, dtype=np.float32)
    sgn = np.where(np.arange(128) % 2 == 0, -1.0, 1.0).astype(
        np.float32).reshape(128, 1)

    wd2_pad = np.zeros((NCORES * MROWS_C, 6400), np.float32)
    wd2_pad[: wd2.shape[0]] = wd2
    bd2_pad = np.zeros(NCORES * MROWS_C, np.float32)
    bd2_pad[: bd2.shape[0]] = bd2

    in_maps = []
    for c in range(NCORES):
        m = {
            "x_cols": x_cols,
            "win_t": np.ascontiguousarray(
                win[MROWS_A * c : MROWS_A * (c + 1)].T),
            "bin_c": _col_major_pad(b_in[MROWS_A * c : MROWS_A * (c + 1)], 13),
            "g_all": g_all,
            "be_all": be_all,
            "wd2_t": np.ascontiguousarray(
                wd2_pad[MROWS_C * c : MROWS_C * (c + 1)].T),
            "bd2_c": _col_major_pad(bd2_pad[MROWS_C * c : MROWS_C * (c + 1)], 5),
            "s0": s0,
            "ident": ident,
            "sgn": sgn,
            "w1c": np.ascontiguousarray(w1t[:, :, :, 64 * c : 64 * (c + 1)]),
            "w2c": np.ascontiguousarray(w2t[:, :, :, 32 * c : 32 * (c + 1)]),
            "g1c": np.ascontiguousarray(g1[64 * c : 64 * (c + 1)].reshape(64, 1)),
            "be1c": np.ascontiguousarray(be1[64 * c : 64 * (c + 1)].reshape(64, 1)),
            "g2c": np.ascontiguousarray(g2[32 * c : 32 * (c + 1)].reshape(32, 1)),
            "be2c": np.ascontiguousarray(be2[32 * c : 32 * (c + 1)].reshape(32, 1)),
        }
        m.update(wts)
        in_maps.append(m)
    return in_maps


LAST_EXEC_NS = None


def kernel(**inputs) -> np.ndarray:
    global LAST_EXEC_NS
    import os

    trace = bool(os.environ.get("KERNEL_TRACE"))
    nc = build_program(T_FULL)
    _split_excess_waits(nc)
    in_maps = _marshal_inputs(inputs)
    res = run_bass_kernel_spmd(nc, in_maps, list(range(NCORES)), trace=trace)
    if res.exec_time_ns is not None:
        LAST_EXEC_NS = res.exec_time_ns
    out = np.asarray(res.results[0]["out"], np.float32)
    return out.reshape(1, T_FULL, N)


if __name__ == "__main__":
    # CoreSim selftest with a short scan (no hardware needed).
    import sys
    import time

    T_test = SCAN_B * 2
    nc = build_program(T_test)
    print("program built", flush=True)

    sys.path.insert(0, "/root/problem")
    import jax
    jax.config.update("jax_platform_name", "cpu")
    import reference

    inputs = reference.setup_inputs()
    inputs = {k: np.asarray(v) for k, v in inputs.items()}
    in_maps = _marshal_inputs(inputs)

    from concourse.bass_interp import MultiCoreSim

    t0 = time.time()
    sim = MultiCoreSim(nc, NCORES)
    for i in range(NCORES):
        for k, v in in_maps[i].items():
            sim.cores[i].tensor(k)[:] = v
    sim.simulate()
    print("sim time", time.time() - t0, flush=True)
    got = np.array(sim.cores[0].tensor("out"))

    # host reference for the short horizon
    w = np.load("/tmp/w.npy")
    s = np.asarray(inputs["start_part"])[-1].astype(np.float32)
    ref = np.empty((T_test, N), np.float32)
    for t in range(T_test):
        s = (np.tanh((s @ w).astype(np.float32)).astype(np.float32) - s).astype(
            np.float32)
        ref[t] = s
    err = np.abs(got - ref)
    rel = np.abs(got - ref) / (np.abs(ref) + 1e-6)
    print("traj absmax err:", err.max(), "rel max:", rel.max())
    print("first rows got:", got[0, :4], "ref:", ref[0, :4])

